# revision 7
# baseline (speedup 1.0000x reference)
"""Trainium2 Bass kernel for nn_ADS_30313879175331.

Pipeline (reference):
  attn-softmax pooling over T -> x *= (1+aw) -> shuffle tokens by perm
  -> Linear(D,D)+GELU -> rearrange (B,T/4,4,D)->(B,T/4,D*4)
  -> gather keep_idx columns -> Linear(D,D) -> (B, T/4, D)

Device strategy (8 cores, shard T):
  * Host folds perm + the (rearrange+keep_idx gather) into pure data layout:
    - tokens are grouped per (core, batch, class r = shuffled_pos % 4)
    - embed weight columns {d : 4d+r in keep_idx} and matching w_down rows are
      pre-selected per class, so the device kernel is fully dense.
  * Per (b,r) tile of 512 tokens (x stored transposed, d on partitions):
      attn1 matmul -> tanh -> logit matmul with w2 replicated over 128 cols
      (logits land broadcast across partitions) -> Exp activation with fused
      row-sum -> embed matmul h = x @ We_r stashed in SBUF (bf16).
  * Per-batch softmax denominator: tiny 8-core AllReduce (hidden under the
    next batch's compute).
  * Stage 2: s = 1 + e/sum ; g = gelu(h*s) ; out = sum_r g_r @ Wd_r + b_down.
  All matmuls bf16 with f32 PSUM accumulation.
"""

import numpy as np
import ml_dtypes

B, T, D, ATTN, R = 4, 16384, 1024, 128, 4
N_CORES = 8
TPC = T // N_CORES          # tokens per core (shuffled-order slice) = 2048
U = TPC // R                # output rows per core per batch = 512
DC = D // 128               # contraction chunks over D = 8
P = 128

_BF16 = ml_dtypes.bfloat16


def _host_prep(x, w_attn1, b_attn1, w_attn2, b_attn2,
               w_embed, b_embed, w_down, b_down, perm, keep_idx):
    """Pure-layout host work: sharding, permutation gather, weight selection."""
    perm = np.asarray(perm).astype(np.int64)
    keep = np.asarray(keep_idx).astype(np.int64)
    x = np.asarray(x, dtype=np.float32)

    # class split of keep_idx (duplicates preserved, order by j)
    cols, rows = [], []
    for r in range(R):
        sel = np.nonzero((keep % R) == r)[0]
        rows.append(sel)                  # indices j into w_down rows
        cols.append(keep[sel] // R)       # embed output columns d
    Kr = [len(c) for c in cols]
    KC = [(k + P - 1) // P for k in Kr]   # 128-chunks per class (may be 0)
    Kp = [kc * P for kc in KC]
    SKC = sum(KC)
    SKP = sum(Kp)
    OFFC = np.concatenate([[0], np.cumsum(KC)]).astype(int)  # chunk offsets

    f32 = np.float32
    we = np.zeros((D, SKP), dtype=f32)
    wd = np.zeros((SKP, D), dtype=f32)
    be = np.zeros((SKP,), dtype=f32)
    for r in range(R):
        o = OFFC[r] * P
        if Kr[r]:
            we[:, o:o + Kr[r]] = np.asarray(w_embed, f32)[:, cols[r]]
            wd[o:o + Kr[r], :] = np.asarray(w_down, f32)[rows[r], :]
            be[o:o + Kr[r]] = np.asarray(b_embed, f32)[cols[r]]
    # bias per (partition, chunk) layout for per-partition activation bias
    be_pc = be.reshape(SKC, P).T.copy()                       # (128, SKC)

    w1 = np.asarray(w_attn1, f32).astype(_BF16)               # (D, ATTN)
    w2r = np.tile(np.asarray(w_attn2, f32).reshape(ATTN, 1), (1, P)).astype(_BF16)
    b1 = np.asarray(b_attn1, f32).reshape(ATTN, 1)
    b2 = np.full((P, 1), float(np.asarray(b_attn2, f32).reshape(-1)[0]), f32)
    bd = np.broadcast_to(np.asarray(b_down, f32), (P, D)).copy()

    # x gather: x_pre[core][b, r, d, u] = x[b, perm[core*TPC + 4u + r], d]
    pidx = perm.reshape(N_CORES, U, R)                        # [core, u, r]
    g = x[:, pidx, :]                                         # (B, cores, U, R, D)
    arr = np.ascontiguousarray(g.transpose(1, 0, 3, 4, 2))    # (cores, B, R, D, U)
    x_pre = arr.astype(_BF16)

    meta = dict(Kr=Kr, KC=KC, Kp=Kp, SKC=SKC, SKP=SKP, OFFC=OFFC)
    weights = dict(
        w1=w1, w2r=w2r, b1=b1, b2=b2, bd=bd,
        we=we.astype(_BF16), wd=wd.astype(_BF16), be=be_pc,
    )
    return x_pre, weights, meta


def _build(meta):
    import concourse.bass as bass
    import concourse.bacc as bacc
    import concourse.mybir as mybir
    import concourse.tile as tile

    dt = mybir.dt
    AF = mybir.ActivationFunctionType
    ALU = mybir.AluOpType
    KC, SKC, SKP, OFFC = meta["KC"], meta["SKC"], meta["SKP"], meta["OFFC"]

    nc = bacc.Bacc(None, target_bir_lowering=False, debug=False,
                   num_devices=N_CORES)

    xp = nc.declare_dram_parameter("x", [B, R, D, U], dt.bfloat16, isOutput=False)
    w1p = nc.declare_dram_parameter("w1", [D, ATTN], dt.bfloat16, isOutput=False)
    w2p = nc.declare_dram_parameter("w2r", [ATTN, P], dt.bfloat16, isOutput=False)
    wep = nc.declare_dram_parameter("we", [D, SKP], dt.bfloat16, isOutput=False)
    wdp = nc.declare_dram_parameter("wd", [SKP, D], dt.bfloat16, isOutput=False)
    bep = nc.declare_dram_parameter("be", [P, SKC], dt.float32, isOutput=False)
    b1p = nc.declare_dram_parameter("b1", [ATTN, 1], dt.float32, isOutput=False)
    b2p = nc.declare_dram_parameter("b2", [P, 1], dt.float32, isOutput=False)
    bdp = nc.declare_dram_parameter("bd", [P, D], dt.float32, isOutput=False)
    outp = nc.declare_dram_parameter("out", [B, U, D], dt.float32, isOutput=True)

    with tile.TileContext(nc) as tc:
        with (
            tc.tile_pool(name="const", bufs=1) as cpool,
            tc.tile_pool(name="xin", bufs=2) as xpool,
            tc.tile_pool(name="acts", bufs=2) as apool,
            tc.tile_pool(name="gts", bufs=2) as gpool,
            tc.tile_pool(name="outs", bufs=2) as opool,
            tc.tile_pool(name="tmps", bufs=3) as tpool,
            tc.tile_pool(name="psA", bufs=4, space="PSUM") as psA,
            tc.tile_pool(name="psO", bufs=2, space="PSUM") as psO,
            tc.tile_pool(name="dram", bufs=1, space="DRAM") as dram,
        ):
            # ---- resident constants ----
            w1_sb = cpool.tile([P, DC, ATTN], dt.bfloat16)
            nc.scalar.dma_start(w1_sb[:], w1p.ap().rearrange("(c p) a -> p c a", p=P))
            w2r_sb = cpool.tile([P, P], dt.bfloat16)
            nc.scalar.dma_start(w2r_sb[:], w2p[:, :])
            we_sb = cpool.tile([P, DC, SKP], dt.bfloat16)
            nc.scalar.dma_start(we_sb[:], wep.ap().rearrange("(c p) k -> p c k", p=P))
            wd_sb = cpool.tile([P, SKC, D], dt.bfloat16)
            nc.scalar.dma_start(wd_sb[:], wdp.ap().rearrange("(c p) n -> p c n", p=P))
            be_sb = cpool.tile([P, SKC], dt.float32)
            nc.scalar.dma_start(be_sb[:], bep[:, :])
            b1_sb = cpool.tile([ATTN, 1], dt.float32)
            nc.scalar.dma_start(b1_sb[:], b1p[:, :])
            b2_sb = cpool.tile([P, 1], dt.float32)
            nc.scalar.dma_start(b2_sb[:], b2p[:, :])
            bd_sb = cpool.tile([P, D], dt.float32)
            nc.scalar.dma_start(bd_sb[:], bdp[:, :])

            e_sb = cpool.tile([P, B, R, U], dt.bfloat16)     # exp(logits), bcast rows
            esum_sb = cpool.tile([P, B * R], dt.float32)     # per-(b,r) local sums
            h_sb = cpool.tile([P, B, SKC, U], dt.bfloat16)   # x @ We (transposed)
            inv_bc = cpool.tile([P, B], dt.float32)          # 1/denominator bcast
            den_sb = cpool.tile([1, 8 * B], dt.float32)

            bounce_in = [dram.tile([1, 8], dt.float32, name=f"cc_in{b}")
                         for b in range(B)]
            bounce_out = [dram.tile([1, 8], dt.float32, addr_space="Shared",
                                    name=f"cc_out{b}") for b in range(B)]

            def stage_a(b):
                """attn logits + exp + embed h for all 4 classes of batch b."""
                for r in range(R):
                    xt = xpool.tile([P, DC, U], dt.bfloat16, tag="xt")
                    nc.sync.dma_start(
                        xt[:], xp[b, r].rearrange("(c p) u -> p c u", p=P))
                    aT = psA.tile([P, U], dt.float32, tag="psA")
                    for c in range(DC):
                        nc.tensor.matmul(aT[:], w1_sb[:, c], xt[:, c],
                                         start=(c == 0), stop=(c == DC - 1))
                    aTs = tpool.tile([P, U], dt.bfloat16, tag="aTs")
                    nc.scalar.activation(aTs[:], aT[:], AF.Tanh, bias=b1_sb[:, 0:1])
                    lps = psA.tile([P, U], dt.float32, tag="psA")
                    nc.tensor.matmul(lps[:], w2r_sb[:], aTs[:], start=True, stop=True)
                    nc.scalar.activation(
                        e_sb[:, b, r], lps[:], AF.Exp, bias=b2_sb[:, 0:1],
                        accum_out=esum_sb[:, b * R + r:b * R + r + 1])
                    for kc in range(KC[r]):
                        ko = (OFFC[r] + kc) * P
                        hp = psA.tile([P, U], dt.float32, tag="psA")
                        for c in range(DC):
                            nc.tensor.matmul(hp[:], we_sb[:, c, ko:ko + P],
                                             xt[:, c],
                                             start=(c == 0), stop=(c == DC - 1))
                        nc.vector.tensor_copy(h_sb[:, b, OFFC[r] + kc], hp[:])

            def issue_collective(b):
                zb = tpool.tile([1, 8], dt.float32, tag="zb")
                nc.gpsimd.memset(zb[:], 0.0)
                nc.vector.tensor_reduce(
                    zb[0:1, 0:1], esum_sb[0:1, b * R:(b + 1) * R],
                    axis=mybir.AxisListType.X, op=ALU.add)
                nc.gpsimd.dma_start(bounce_in[b][:], zb[:])
                nc.gpsimd.collective_compute(
                    "AllReduce", ALU.add,
                    ins=[bounce_in[b][:]],
                    outs=[bounce_out[b][:]],
                    replica_groups=[list(range(N_CORES))],
                )

            def stage2(b):
                nc.scalar.dma_start(den_sb[0:1, 8 * b:8 * (b + 1)],
                                    bounce_out[b][:])
                nc.vector.reciprocal(den_sb[0:1, 8 * b + 4:8 * b + 5],
                                     den_sb[0:1, 8 * b:8 * b + 1])
                nc.gpsimd.partition_broadcast(inv_bc[:, b:b + 1],
                                              den_sb[0:1, 8 * b + 4:8 * b + 5])
                gT = gpool.tile([P, SKC, U], dt.bfloat16, tag="gT")
                for r in range(R):
                    if KC[r] == 0:
                        continue
                    st = tpool.tile([P, U], dt.float32, tag="st")
                    nc.scalar.activation(st[:], e_sb[:, b, r], AF.Identity,
                                         bias=1.0, scale=inv_bc[:, b:b + 1])
                    for kc in range(KC[r]):
                        ci = OFFC[r] + kc
                        tmp = tpool.tile([P, U], dt.float32, tag="tmp")
                        nc.vector.tensor_tensor(tmp[:], h_sb[:, b, ci], st[:],
                                                ALU.mult)
                        nc.scalar.activation(gT[:, ci], tmp[:], AF.Gelu,
                                             bias=be_sb[:, ci:ci + 1])
                for u in range(U // P):
                    ob = opool.tile([P, D], dt.float32, tag="ob")
                    po = [psO.tile([P, D // 2], dt.float32, tag=f"psO{dn}",
                                   name=f"po{dn}")
                          for dn in range(2)]
                    i = 0
                    for r in range(R):
                        for kc in range(KC[r]):
                            ci = OFFC[r] + kc
                            # same stationary lhsT for both output halves
                            for dn in range(2):
                                nc.tensor.matmul(
                                    po[dn][:], gT[:, ci, u * P:(u + 1) * P],
                                    wd_sb[:, ci, dn * (D // 2):(dn + 1) * (D // 2)],
                                    start=(i == 0), stop=(i == SKC - 1))
                            i += 1
                    for dn in range(2):
                        nc.vector.tensor_tensor(
                            ob[:, dn * (D // 2):(dn + 1) * (D // 2)], po[dn][:],
                            bd_sb[:, dn * (D // 2):(dn + 1) * (D // 2)], ALU.add)
                    nc.sync.dma_start(outp[b, u * P:(u + 1) * P, :], ob[:])

            # all stage-A work first (PE never waits on a collective), then
            # the stage-2 phases: collective b completes while A(b+1..) runs
            for b in range(B):
                stage_a(b)
                issue_collective(b)
            for b in range(B):
                stage2(b)

    nc.compile()
    return nc


def _run(inputs, trace=False):
    from concourse.bass_utils import run_bass_kernel_spmd

    x_pre, weights, meta = _host_prep(**inputs)
    nc = _build(meta)
    in_maps = [dict(x=np.ascontiguousarray(x_pre[c]), **weights)
               for c in range(N_CORES)]
    res = run_bass_kernel_spmd(nc, in_maps, core_ids=list(range(N_CORES)),
                               trace=trace)
    out = np.empty((B, T // R, D), dtype=np.float32)
    for c in range(N_CORES):
        out[:, c * U:(c + 1) * U, :] = res.results[c]["out"]
    return out, res


def kernel(**inputs):
    out, _ = _run(inputs, trace=False)
    return out
